# revision 7
# baseline (speedup 1.0000x reference)
"""AccentEncoder Trainium2 kernel.

Pipeline per core (2 batch images of the 16, data-parallel over 8 cores):
  conv1 (80->256, k5 p2) + folded BN + ReLU
  conv2 (256->256, k5 p2) + folded BN + ReLU
  downsample conv (256->128, k24 s12 p6)  -> e_a_unq (output 1)
  1x1 conv (128->16)                      -> z
  RVQ: 8 sequential argmin stages         -> indices (output 2)

Convs are computed as sums of shifted matmuls (tap-decomposition), BN is
folded into the conv weights on the host.  The RVQ argmin uses the identity
argmin_k ||r-c_k||^2 == argmax_k (r.c_k - ||c_k||^2/2), computed with one
matmul per (stage, position-tile) whose contraction includes an all-ones row
so the -||c_k||^2/2 term rides along; DVE max/max_index extracts the winner
and an indirect DMA gathers the selected codes for the residual update.
"""

import os
import sys

import numpy as np

try:
    import concourse  # noqa: F401
except ImportError:
    sys.path.insert(0, "/opt/trn_rl_repo")

NCORES = 8
B, CIN, T = 16, 80, 4096
BPC = B // NCORES          # images per core
C = 256                    # conv1/conv2 channels
K1 = 5                     # conv1/conv2 kernel width
STRIDE, KD = 12, 24        # downsample conv
EMB, VQD, NQ, K = 128, 16, 8, 256
TD = (T + 2 * (STRIDE // 2) - KD) // STRIDE + 1   # 341
TPOS = BPC * TD            # positions per core
PAD = 6                    # unified halo (>= conv pad 2 and downsample pad 6)
PADT = T + 2 * PAD         # 4108
BN_EPS = 1e-5
CT = 512                   # conv column-tile width
NCT = T // CT              # 8 column tiles
NPT = (TPOS + 127) // 128  # RVQ position tiles (6)

# matmul dtype mode: "fp32" (exact, 4 cyc/row) or "fp32r" (1 cyc/row @ N>=256)
MM_MODE = os.environ.get("ACCENT_MM_MODE", "fp32")

_PROGRAM_CACHE = {}


def _build_program(mm_mode):
    from concourse import bacc, bass, mybir, tile
    from concourse.masks import make_identity

    fp32 = mybir.dt.float32
    i32 = mybir.dt.int32
    u32 = mybir.dt.uint32

    # Bacc (not raw Bass): its compile pass moves matmul waits onto ldweights
    # and splits over-limit sync waits, which conv accumulation groups need.
    nc = bacc.Bacc("TRN2")

    mel_d = nc.dram_tensor("mel", [BPC, CIN, T], fp32, kind="ExternalInput")
    w1t_d = nc.dram_tensor("w1t", [K1, CIN, C], fp32, kind="ExternalInput")
    b1_d = nc.dram_tensor("b1f", [2, 128], fp32, kind="ExternalInput")
    w2t_d = nc.dram_tensor("w2t", [K1, 2, 128, C], fp32, kind="ExternalInput")
    b2_d = nc.dram_tensor("b2f", [2, 128], fp32, kind="ExternalInput")
    wdt_d = nc.dram_tensor("wdt", [KD, 2, 128, EMB], fp32, kind="ExternalInput")
    bd_d = nc.dram_tensor("bdf", [EMB], fp32, kind="ExternalInput")
    wpt_d = nc.dram_tensor("wpt", [EMB, VQD], fp32, kind="ExternalInput")
    bp_d = nc.dram_tensor("bpf", [VQD], fp32, kind="ExternalInput")
    cbt_d = nc.dram_tensor("cbt", [VQD + 1, NQ * K], fp32, kind="ExternalInput")
    cbf_d = nc.dram_tensor("cbf", [NQ * K, VQD], fp32, kind="ExternalInput")

    e_out_d = nc.dram_tensor("e_out", [BPC, EMB, TD], fp32, kind="ExternalOutput")
    idx_out_d = nc.dram_tensor("idx_out", [TPOS, NQ], i32, kind="ExternalOutput")

    fast = mm_mode == "fp32r"

    def mm(out, lhsT, rhs, start, stop):
        if fast:
            lhsT = lhsT.bitcast(mybir.dt.float32r)
            rhs = rhs.bitcast(mybir.dt.float32r)
        nc.tensor.matmul(out, lhsT, rhs, start=start, stop=stop)

    with tile.TileContext(nc) as tc:
        with (
            tc.tile_pool(name="const", bufs=1) as cpool,
            tc.tile_pool(name="x0p", bufs=2) as x0pool,
            tc.tile_pool(name="x1p", bufs=1) as x1pool,
            tc.tile_pool(name="x2p", bufs=1) as x2pool,
            tc.tile_pool(name="work", bufs=3) as wpool,
            tc.tile_pool(name="psA", bufs=2, space="PSUM") as psA,
            tc.tile_pool(name="psB", bufs=2, space="PSUM") as psB,
            tc.tile_pool(name="psG", bufs=2, space="PSUM") as psG,
            tc.tile_pool(name="psM", bufs=2, space="PSUM") as psM,
        ):
            # ---- static weights / constants -------------------------------
            # single DMA per weight tensor (fan-out over many DMA queues blows
            # the per-instruction sync-wait limit on downstream matmuls)
            w1t_sb = cpool.tile([CIN, K1 * C], fp32, tag="w1t")
            nc.sync.dma_start(
                w1t_sb[:].rearrange("p (k c) -> p k c", k=K1),
                w1t_d[:].transpose([1, 0, 2]))
            w2t_sb = cpool.tile([128, K1 * 2 * C], fp32, tag="w2t")
            nc.sync.dma_start(
                w2t_sb[:].rearrange("p (k g c) -> p k g c", k=K1, g=2),
                w2t_d[:].transpose([2, 0, 1, 3]))
            wdt_sb = cpool.tile([128, KD * 2 * EMB], fp32, tag="wdt")
            nc.sync.dma_start(
                wdt_sb[:].rearrange("p (k g c) -> p k g c", k=KD, g=2),
                wdt_d[:].transpose([2, 0, 1, 3]))
            wpt_sb = cpool.tile([EMB, VQD], fp32, tag="wpt")
            nc.sync.dma_start(wpt_sb[:], wpt_d[:])
            b1_sb = cpool.tile([128, 2], fp32, tag="b1")
            b2_sb = cpool.tile([128, 2], fp32, tag="b2")
            nc.sync.dma_start(b1_sb[:], b1_d[:].transpose([1, 0]))
            nc.sync.dma_start(b2_sb[:], b2_d[:].transpose([1, 0]))
            bd_sb = cpool.tile([EMB, 1], fp32, tag="bd")
            nc.sync.dma_start(bd_sb[:], bd_d[:].unsqueeze(1))
            bp_sb = cpool.tile([VQD, 1], fp32, tag="bp")
            nc.sync.dma_start(bp_sb[:], bp_d[:].unsqueeze(1))
            cbt_sb = cpool.tile([VQD + 1, NQ * K], fp32, tag="cbt")
            nc.sync.dma_start(cbt_sb[:], cbt_d[:])
            ident = cpool.tile([128, 128], fp32, tag="ident")
            make_identity(nc, ident[:])

            e_sb = cpool.tile([EMB, TPOS], fp32, tag="e_sb")
            rts = [cpool.tile([VQD + 1, 128], fp32, tag=f"rt{i}", name=f"rt{i}")
                   for i in range(NPT)]
            idxs = [cpool.tile([128, NQ], i32, tag=f"idx{i}", name=f"idx{i}")
                    for i in range(NPT)]

            relu = mybir.ActivationFunctionType.Relu

            # ---- conv stack, one image at a time --------------------------
            for img in range(BPC):
                x0 = x0pool.tile([CIN, PADT], fp32, tag="x0")
                nc.gpsimd.memset(x0[:, 0:PAD], 0.0)
                nc.gpsimd.memset(x0[:, T + PAD:PADT], 0.0)
                nc.sync.dma_start(x0[:, PAD:T + PAD], mel_d[img])

                x1c = [x1pool.tile([128, PADT], fp32, tag=f"x1_{c}", name=f"x1_{c}")
                       for c in range(2)]
                x2c = [x2pool.tile([128, PADT], fp32, tag=f"x2_{c}", name=f"x2_{c}")
                       for c in range(2)]
                for cc in range(2):
                    nc.gpsimd.memset(x1c[cc][:, 0:PAD], 0.0)
                    nc.gpsimd.memset(x1c[cc][:, T + PAD:PADT], 0.0)
                    nc.gpsimd.memset(x2c[cc][:, 0:PAD], 0.0)
                    nc.gpsimd.memset(x2c[cc][:, T + PAD:PADT], 0.0)

                # conv1: K=80, 5 taps, out channels 2 halves
                for h in range(2):
                    for ct in range(NCT):
                        ps = psA.tile([128, CT], fp32, tag="c1")
                        t0 = PAD - 2 + ct * CT
                        for dk in range(K1):
                            mm(ps[:], w1t_sb[:, dk * C + h * 128: dk * C + h * 128 + 128],
                               x0[:, t0 + dk: t0 + dk + CT],
                               start=(dk == 0), stop=(dk == K1 - 1))
                        nc.scalar.activation(
                            x1c[h][:, PAD + ct * CT: PAD + (ct + 1) * CT],
                            ps[:], relu, bias=b1_sb[:, h:h + 1])

                # conv2: K=256 (2 chunks) x 5 taps
                for h in range(2):
                    for ct in range(NCT):
                        ps = psB.tile([128, CT], fp32, tag="c2")
                        t0 = PAD - 2 + ct * CT
                        n = 0
                        for dk in range(K1):
                            for cc in range(2):
                                o = (dk * 2 + cc) * C + h * 128
                                mm(ps[:], w2t_sb[:, o:o + 128],
                                   x1c[cc][:, t0 + dk: t0 + dk + CT],
                                   start=(n == 0), stop=(n == 2 * K1 - 1))
                                n += 1
                        nc.scalar.activation(
                            x2c[h][:, PAD + ct * CT: PAD + (ct + 1) * CT],
                            ps[:], relu, bias=b2_sb[:, h:h + 1])

                # downsample conv: out (128, 341), K=256x24 taps, strided rhs
                eps = psM.tile([EMB, TD], fp32, tag="misc")
                n = 0
                for j in range(KD):
                    for cc in range(2):
                        o = (j * 2 + cc) * EMB
                        mm(eps[:], wdt_sb[:, o:o + EMB],
                           x2c[cc][:, j: j + STRIDE * (TD - 1) + 1: STRIDE],
                           start=(n == 0), stop=(n == 2 * KD - 1))
                        n += 1
                nc.vector.tensor_scalar_add(
                    e_sb[:, img * TD:(img + 1) * TD], eps[:], bd_sb[:, 0:1])
                nc.sync.dma_start(e_out_d[img], e_sb[:, img * TD:(img + 1) * TD])

            # ---- 1x1 conv -> residual tiles (VQ dim on partitions) --------
            for i in range(NPT):
                p0 = i * 128
                pn = min(128, TPOS - p0)
                zps = psM.tile([VQD, 128], fp32, tag="misc")
                # z = wp.T @ e  (K=128, M=16, N=pn); keep full fp32
                nc.tensor.matmul(zps[:, :pn], wpt_sb[:], e_sb[:, p0:p0 + pn],
                                 start=True, stop=True)
                # row 16 must be all-ones (rides the -||c||^2/2 term through
                # the score matmul); engines can't address partition 16 alone,
                # so fill the whole tile then overwrite rows 0..15.
                nc.gpsimd.memset(rts[i][:], 1.0)
                nc.vector.tensor_scalar_add(rts[i][0:VQD, :pn], zps[:, :pn],
                                            bp_sb[:, 0:1])

            # ---- RVQ: 8 sequential stages ---------------------------------
            for q in range(NQ):
                for i in range(NPT):
                    p0 = i * 128
                    pn = min(128, TPOS - p0)
                    rt = rts[i]
                    gps = psG.tile([128, K], fp32, tag="g")
                    # scores g[t,k] = r_t . c_k - ||c_k||^2/2 (exact fp32)
                    nc.tensor.matmul(gps[:pn], rt[:, :pn],
                                     cbt_sb[:, q * K:(q + 1) * K],
                                     start=True, stop=True)
                    gsb = wpool.tile([128, K], fp32, tag="gsb")
                    nc.scalar.copy(gsb[:pn], gps[:pn])
                    m8 = wpool.tile([128, 8], fp32, tag="m8")
                    nc.vector.max(m8[:pn], gsb[:pn])
                    i8 = wpool.tile([128, 8], u32, tag="i8")
                    nc.vector.max_index(i8[:pn], m8[:pn], gsb[:pn])
                    nc.vector.tensor_copy(idxs[i][:pn, q:q + 1], i8[:pn, 0:1])
                    if q < NQ - 1:
                        # gather selected codes, transpose, subtract residual
                        ib = wpool.tile([128, 1], u32, tag="ib")
                        nc.vector.tensor_scalar_add(ib[:pn], i8[:pn, 0:1], q * K)
                        sel = wpool.tile([128, VQD], fp32, tag="sel")
                        nc.gpsimd.indirect_dma_start(
                            out=sel[:pn], out_offset=None, in_=cbf_d[:],
                            in_offset=bass.IndirectOffsetOnAxis(
                                ap=ib[:pn, 0:1], axis=0))
                        stps = psM.tile([VQD, 128], fp32, tag="misc")
                        nc.tensor.transpose(stps[:, :pn], sel[:pn, :],
                                            ident[:pn, :pn])
                        nc.vector.tensor_sub(rt[0:VQD, :pn], rt[0:VQD, :pn],
                                             stps[:, :pn])
            for i in range(NPT):
                p0 = i * 128
                pn = min(128, TPOS - p0)
                nc.sync.dma_start(idx_out_d[p0:p0 + pn, :], idxs[i][:pn, :])

    nc.finalize()
    return nc


def _host_prep(inputs):
    f = lambda k: np.asarray(inputs[k], np.float32)
    w1, b1, g1, be1, m1, v1 = (f(k) for k in ("w1", "b1", "g1", "be1", "m1", "v1"))
    w2, b2, g2, be2, m2, v2 = (f(k) for k in ("w2", "b2", "g2", "be2", "m2", "v2"))
    wd, bd, wp, bp, cb = (f(k) for k in ("wd", "bd", "wp", "bp", "codebooks"))

    s1 = (g1 / np.sqrt(v1 + BN_EPS)).astype(np.float32)
    w1f = w1 * s1[:, None, None]
    b1f = ((b1 - m1) * s1 + be1).astype(np.float32)
    s2 = (g2 / np.sqrt(v2 + BN_EPS)).astype(np.float32)
    w2f = w2 * s2[:, None, None]
    b2f = ((b2 - m2) * s2 + be2).astype(np.float32)

    w1t = np.ascontiguousarray(np.transpose(w1f, (2, 1, 0)))          # (5,80,256)
    w2t = np.ascontiguousarray(
        np.transpose(w2f, (2, 1, 0)).reshape(K1, 2, 128, C))          # (5,2,128,256)
    wdt = np.ascontiguousarray(
        np.transpose(wd, (2, 1, 0)).reshape(KD, 2, 128, EMB))         # (24,2,128,128)
    wpt = np.ascontiguousarray(wp[:, :, 0].T)                         # (128,16)

    cbt = np.empty((VQD + 1, NQ * K), np.float32)
    for q in range(NQ):
        cbt[:VQD, q * K:(q + 1) * K] = cb[q].T
        cbt[VQD, q * K:(q + 1) * K] = -0.5 * np.sum(cb[q] * cb[q], axis=-1)
    cbf = np.ascontiguousarray(cb.reshape(NQ * K, VQD))

    shared = {
        "w1t": w1t, "b1f": b1f.reshape(2, 128),
        "w2t": w2t, "b2f": b2f.reshape(2, 128),
        "wdt": wdt, "bdf": bd, "wpt": wpt, "bpf": bp,
        "cbt": cbt, "cbf": cbf,
    }
    return shared


def kernel(**inputs):
    from concourse.bass_utils import run_bass_kernel_spmd

    mm_mode = MM_MODE
    if mm_mode not in _PROGRAM_CACHE:
        _PROGRAM_CACHE[mm_mode] = _build_program(mm_mode)
    nc = _PROGRAM_CACHE[mm_mode]

    shared = _host_prep(inputs)
    mel = np.asarray(inputs["mel_spectrogram"], np.float32)

    in_maps = [dict(shared, mel=np.ascontiguousarray(mel[c * BPC:(c + 1) * BPC]))
               for c in range(NCORES)]

    res = run_bass_kernel_spmd(nc, in_maps, list(range(NCORES)))
    results = res.results

    e_full = np.empty((B, EMB, TD), np.float32)
    idx_full = np.empty((NQ, B, TD), np.int32)
    for c in range(NCORES):
        e_full[c * BPC:(c + 1) * BPC] = results[c]["e_out"]
        idx = results[c]["idx_out"]                  # (TPOS, NQ)
        idx_full[:, c * BPC:(c + 1) * BPC, :] = (
            idx.T.reshape(NQ, BPC, TD))
    return e_full, idx_full


# revision 13
# speedup vs baseline: 1.4138x; 1.4138x over previous
"""AccentEncoder Trainium2 kernel.

Pipeline per core (2 batch images of the 16, data-parallel over 8 cores):
  conv1 (80->256, k5 p2) + folded BN + ReLU
  conv2 (256->256, k5 p2) + folded BN + ReLU
  downsample conv (256->128, k24 s12 p6)  -> e_a_unq (output 1)
  1x1 conv (128->16)                      -> z
  RVQ: 8 sequential argmin stages         -> indices (output 2)

Convs are computed as sums of shifted matmuls (tap-decomposition), BN is
folded into the conv weights on the host.  The RVQ argmin uses the identity
argmin_k ||r-c_k||^2 == argmax_k (r.c_k - ||c_k||^2/2), computed with one
matmul per (stage, position-tile) whose contraction includes an all-ones row
so the -||c_k||^2/2 term rides along; DVE max/max_index extracts the winner
and an indirect DMA gathers the selected codes for the residual update.
"""

import os
import sys

import numpy as np

try:
    import concourse  # noqa: F401
except ImportError:
    sys.path.insert(0, "/opt/trn_rl_repo")

NCORES = 8
B, CIN, T = 16, 80, 4096
BPC = B // NCORES          # images per core
C = 256                    # conv1/conv2 channels
K1 = 5                     # conv1/conv2 kernel width
STRIDE, KD = 12, 24        # downsample conv
EMB, VQD, NQ, K = 128, 16, 8, 256
TD = (T + 2 * (STRIDE // 2) - KD) // STRIDE + 1   # 341
TPOS = BPC * TD            # positions per core
PAD = 6                    # unified halo (>= conv pad 2 and downsample pad 6)
PADT = T + 2 * PAD         # 4108
BN_EPS = 1e-5
CT = 512                   # conv column-tile width
NCT = T // CT              # 8 column tiles
NPT = (TPOS + 127) // 128  # RVQ position tiles (6)

# matmul mode: "fp16x2" (split-2 fp16, 3 cyc/row, fp32-class accuracy),
# "fp32" (exact, ~8 cyc/row), "fp32r" (fast but flips VQ indices -- unsafe)
MM_MODE = os.environ.get("ACCENT_MM_MODE", "fp16x2")

_PROGRAM_CACHE = {}


def _build_program(mm_mode):
    from concourse import bacc, bass, mybir, tile
    from concourse.masks import make_identity

    fp32 = mybir.dt.float32
    i32 = mybir.dt.int32
    u32 = mybir.dt.uint32

    # Bacc (not raw Bass): its compile pass moves matmul waits onto ldweights
    # and splits over-limit sync waits, which conv accumulation groups need.
    nc = bacc.Bacc("TRN2")

    fast = mm_mode == "fp32r"
    # conv-path operand dtype: float32r tiles make the producer chain
    # (DMA / ACT epilogue) emit fp32r-rounded data, which the BIR verifier
    # requires for fp32r matmuls. 4x PE throughput at ~1.6e-4 rel err.
    xdt = mybir.dt.float32r if fast else fp32

    mel_d = nc.dram_tensor("mel", [BPC, CIN, PADT], xdt, kind="ExternalInput")
    w1t_d = nc.dram_tensor("w1t", [K1, CIN, C], xdt, kind="ExternalInput")
    b1_d = nc.dram_tensor("b1f", [2, 128], fp32, kind="ExternalInput")
    w2t_d = nc.dram_tensor("w2t", [K1, 2, 128, C], xdt, kind="ExternalInput")
    b2_d = nc.dram_tensor("b2f", [2, 128], fp32, kind="ExternalInput")
    wdt_d = nc.dram_tensor("wdt", [KD, 2, 128, EMB], xdt, kind="ExternalInput")
    bd_d = nc.dram_tensor("bdf", [EMB], fp32, kind="ExternalInput")
    wpt_d = nc.dram_tensor("wpt", [EMB, VQD], fp32, kind="ExternalInput")
    bp_d = nc.dram_tensor("bpf", [VQD], fp32, kind="ExternalInput")
    cbt_d = nc.dram_tensor("cbt", [VQD + 1, NQ * K], fp32, kind="ExternalInput")
    cbf_d = nc.dram_tensor("cbf", [NQ * K, VQD], fp32, kind="ExternalInput")

    e_out_d = nc.dram_tensor("e_out", [BPC, EMB, TD], fp32, kind="ExternalOutput")
    idx_out_d = nc.dram_tensor("idx_out", [TPOS, NQ], i32, kind="ExternalOutput")

    def mm(out, lhsT, rhs, start, stop, force_fp32=False):
        if fast and force_fp32:
            # fp32r matmuls reject strided moving APs (s3d3_mm_fp32r
            # restrictions); the strided-rhs downsample conv runs in fp32.
            lhsT = lhsT.bitcast(fp32)
            rhs = rhs.bitcast(fp32)
        nc.tensor.matmul(out, lhsT, rhs, start=start, stop=stop)

    with tile.TileContext(nc) as tc:
        with (
            tc.tile_pool(name="const", bufs=1) as cpool,
            tc.tile_pool(name="x0p", bufs=2) as x0pool,
            tc.tile_pool(name="x1p", bufs=1) as x1pool,
            tc.tile_pool(name="x2p", bufs=1) as x2pool,
            tc.tile_pool(name="work", bufs=3) as wpool,
            tc.tile_pool(name="psA", bufs=2, space="PSUM") as psA,
            tc.tile_pool(name="psB", bufs=2, space="PSUM") as psB,
            tc.tile_pool(name="psG", bufs=2, space="PSUM") as psG,
            tc.tile_pool(name="psM", bufs=2, space="PSUM") as psM,
        ):
            # ---- static weights / constants -------------------------------
            # single DMA per weight tensor (fan-out over many DMA queues blows
            # the per-instruction sync-wait limit on downstream matmuls)
            w1t_sb = cpool.tile([CIN, K1 * C], xdt, tag="w1t")
            nc.sync.dma_start(
                w1t_sb[:].rearrange("p (k c) -> p k c", k=K1),
                w1t_d[:].transpose([1, 0, 2]))
            w2t_sb = cpool.tile([128, K1 * 2 * C], xdt, tag="w2t")
            nc.sync.dma_start(
                w2t_sb[:].rearrange("p (k g c) -> p k g c", k=K1, g=2),
                w2t_d[:].transpose([2, 0, 1, 3]))
            wdt_sb = cpool.tile([128, KD * 2 * EMB], xdt, tag="wdt")
            nc.sync.dma_start(
                wdt_sb[:].rearrange("p (k g c) -> p k g c", k=KD, g=2),
                wdt_d[:].transpose([2, 0, 1, 3]))
            wpt_sb = cpool.tile([EMB, VQD], fp32, tag="wpt")
            nc.sync.dma_start(wpt_sb[:], wpt_d[:])
            b1_sb = cpool.tile([128, 2], fp32, tag="b1")
            b2_sb = cpool.tile([128, 2], fp32, tag="b2")
            nc.sync.dma_start(b1_sb[:], b1_d[:].transpose([1, 0]))
            nc.sync.dma_start(b2_sb[:], b2_d[:].transpose([1, 0]))
            bd_sb = cpool.tile([EMB, 1], fp32, tag="bd")
            nc.sync.dma_start(bd_sb[:], bd_d[:].unsqueeze(1))
            bp_sb = cpool.tile([VQD, 1], fp32, tag="bp")
            nc.sync.dma_start(bp_sb[:], bp_d[:].unsqueeze(1))
            cbt_sb = cpool.tile([VQD + 1, NQ * K], fp32, tag="cbt")
            nc.sync.dma_start(cbt_sb[:], cbt_d[:])
            ident = cpool.tile([128, 128], fp32, tag="ident")
            make_identity(nc, ident[:])

            e_sb = cpool.tile([EMB, TPOS], fp32, tag="e_sb")
            rts = [cpool.tile([VQD + 1, 128], fp32, tag=f"rt{i}", name=f"rt{i}")
                   for i in range(NPT)]
            idxs = [cpool.tile([128, NQ], i32, tag=f"idx{i}", name=f"idx{i}")
                    for i in range(NPT)]

            relu = mybir.ActivationFunctionType.Relu

            # ---- conv stack, one image at a time --------------------------
            for img in range(BPC):
                x0 = x0pool.tile([CIN, PADT], xdt, tag="x0")
                nc.sync.dma_start(x0[:], mel_d[img])  # host pre-pads halo

                x1c = [x1pool.tile([128, PADT], xdt, tag=f"x1_{c}", name=f"x1_{c}")
                       for c in range(2)]
                x2c = [x2pool.tile([128, PADT], xdt, tag=f"x2_{c}", name=f"x2_{c}")
                       for c in range(2)]
                # pad zeroing on ACT (gpsimd Memset can't emit fp32r, and
                # keeping all x1/x2 writers on one engine keeps matmul waits
                # under the HW sync-wait limit): out = ident*0.0
                for cc in range(2):
                    for xt in (x1c[cc], x2c[cc]):
                        nc.scalar.mul(xt[:, 0:PAD], ident[:, 0:PAD], 0.0)
                        nc.scalar.mul(xt[:, T + PAD:PADT],
                                      ident[:, 0:PAD], 0.0)

                # conv1: K=80, 5 taps, out channels 2 halves
                for h in range(2):
                    for ct in range(NCT):
                        ps = psA.tile([128, CT], fp32, tag="c1")
                        t0 = PAD - 2 + ct * CT
                        for dk in range(K1):
                            mm(ps[:], w1t_sb[:, dk * C + h * 128: dk * C + h * 128 + 128],
                               x0[:, t0 + dk: t0 + dk + CT],
                               start=(dk == 0), stop=(dk == K1 - 1))
                        nc.scalar.activation(
                            x1c[h][:, PAD + ct * CT: PAD + (ct + 1) * CT],
                            ps[:], relu, bias=b1_sb[:, h:h + 1])

                # conv2: K=256 (2 chunks) x 5 taps
                for h in range(2):
                    for ct in range(NCT):
                        ps = psB.tile([128, CT], fp32, tag="c2")
                        t0 = PAD - 2 + ct * CT
                        n = 0
                        for dk in range(K1):
                            for cc in range(2):
                                o = (dk * 2 + cc) * C + h * 128
                                mm(ps[:], w2t_sb[:, o:o + 128],
                                   x1c[cc][:, t0 + dk: t0 + dk + CT],
                                   start=(n == 0), stop=(n == 2 * K1 - 1))
                                n += 1
                        nc.scalar.activation(
                            x2c[h][:, PAD + ct * CT: PAD + (ct + 1) * CT],
                            ps[:], relu, bias=b2_sb[:, h:h + 1])

                # downsample conv: out (128, 341), K=256x24 taps, strided rhs
                eps = psM.tile([EMB, TD], fp32, tag="misc")
                n = 0
                for j in range(KD):
                    for cc in range(2):
                        o = (j * 2 + cc) * EMB
                        mm(eps[:], wdt_sb[:, o:o + EMB],
                           x2c[cc][:, j: j + STRIDE * (TD - 1) + 1: STRIDE],
                           start=(n == 0), stop=(n == 2 * KD - 1),
                           force_fp32=True)
                        n += 1
                nc.vector.tensor_scalar_add(
                    e_sb[:, img * TD:(img + 1) * TD], eps[:], bd_sb[:, 0:1])
                nc.sync.dma_start(e_out_d[img], e_sb[:, img * TD:(img + 1) * TD])

            # ---- 1x1 conv -> residual tiles (VQ dim on partitions) --------
            for i in range(NPT):
                p0 = i * 128
                pn = min(128, TPOS - p0)
                zps = psM.tile([VQD, 128], fp32, tag="misc")
                # z = wp.T @ e  (K=128, M=16, N=pn); keep full fp32
                nc.tensor.matmul(zps[:, :pn], wpt_sb[:], e_sb[:, p0:p0 + pn],
                                 start=True, stop=True)
                # row 16 must be all-ones (rides the -||c||^2/2 term through
                # the score matmul); engines can't address partition 16 alone,
                # so fill the whole tile then overwrite rows 0..15.
                nc.gpsimd.memset(rts[i][:], 1.0)
                nc.vector.tensor_scalar_add(rts[i][0:VQD, :pn], zps[:, :pn],
                                            bp_sb[:, 0:1])

            # ---- RVQ: 8 sequential stages ---------------------------------
            for q in range(NQ):
                for i in range(NPT):
                    p0 = i * 128
                    pn = min(128, TPOS - p0)
                    rt = rts[i]
                    gps = psG.tile([128, K], fp32, tag="g")
                    # scores g[t,k] = r_t . c_k - ||c_k||^2/2 (exact fp32)
                    nc.tensor.matmul(gps[:pn], rt[:, :pn],
                                     cbt_sb[:, q * K:(q + 1) * K],
                                     start=True, stop=True)
                    gsb = wpool.tile([128, K], fp32, tag="gsb")
                    nc.scalar.copy(gsb[:pn], gps[:pn])
                    m8 = wpool.tile([128, 8], fp32, tag="m8")
                    nc.vector.max(m8[:pn], gsb[:pn])
                    i8 = wpool.tile([128, 8], u32, tag="i8")
                    nc.vector.max_index(i8[:pn], m8[:pn], gsb[:pn])
                    nc.vector.tensor_copy(idxs[i][:pn, q:q + 1], i8[:pn, 0:1])
                    if q < NQ - 1:
                        # gather selected codes, transpose, subtract residual
                        ib = wpool.tile([128, 1], u32, tag="ib")
                        nc.vector.tensor_scalar_add(ib[:pn], i8[:pn, 0:1], q * K)
                        sel = wpool.tile([128, VQD], fp32, tag="sel")
                        nc.gpsimd.indirect_dma_start(
                            out=sel[:pn], out_offset=None, in_=cbf_d[:],
                            in_offset=bass.IndirectOffsetOnAxis(
                                ap=ib[:pn, 0:1], axis=0))
                        stps = psM.tile([VQD, 128], fp32, tag="misc")
                        nc.tensor.transpose(stps[:, :pn], sel[:pn, :],
                                            ident[:pn, :pn])
                        nc.vector.tensor_sub(rt[0:VQD, :pn], rt[0:VQD, :pn],
                                             stps[:, :pn])
            for i in range(NPT):
                p0 = i * 128
                pn = min(128, TPOS - p0)
                nc.sync.dma_start(idx_out_d[p0:p0 + pn, :], idxs[i][:pn, :])

    nc.finalize()
    return nc


# ---- fp16 split-2 builder ------------------------------------------------
"""

Every conv matmul runs as 3 fp16 matmuls (xh.wh + xh.wl + xl.wh) where
x = xh + xl is the exact fp16 hi/lo decomposition: ~2^-21 relative error
(fp32-class, verified 4.9e-7 on HW) at 3x1 cyc/row instead of fp32's ~8.

conv2's epilogue writes its output directly in a phase-packed layout
x2p[c, r, m] = x2[c, 12*(m-1)+r], which turns the stride-12 downsample conv
into contiguous-rhs matmuls (strided fp16 moving APs run ~6.5x slower).
RVQ stays exact fp32 so the argmin indices match the reference bit-for-bit.
"""

CT2 = 504                     # conv2 tile width (multiple of 12)
N2FULL = T // CT2             # 8 full tiles
REM2 = T - N2FULL * CT2       # 64
MB = 344                      # packed m size (m_phys = m+1, m in [-1, 342])
NLT = 3                       # RVQ position tiles per image (128,128,85)


def build_fp16():
    from concourse import bacc, bass, mybir, tile
    from concourse.masks import make_identity

    fp32 = mybir.dt.float32
    f16 = mybir.dt.float16
    i32 = mybir.dt.int32
    u32 = mybir.dt.uint32

    nc = bacc.Bacc("TRN2")

    melh_d = nc.dram_tensor("melh", [BPC, CIN, PADT], f16, kind="ExternalInput")
    mell_d = nc.dram_tensor("mell", [BPC, CIN, PADT], f16, kind="ExternalInput")
    w1t_d = nc.dram_tensor("w1t", [2, K1, CIN, C], f16, kind="ExternalInput")
    b1_d = nc.dram_tensor("b1f", [2, 128], fp32, kind="ExternalInput")
    w2t_d = nc.dram_tensor("w2t", [2, K1, 2, 128, C], f16, kind="ExternalInput")
    b2_d = nc.dram_tensor("b2f", [2, 128], fp32, kind="ExternalInput")
    wdt_d = nc.dram_tensor("wdt", [2, KD, 2, 128, EMB], f16, kind="ExternalInput")
    bd_d = nc.dram_tensor("bdf", [EMB], fp32, kind="ExternalInput")
    wpt_d = nc.dram_tensor("wpt", [EMB, VQD], fp32, kind="ExternalInput")
    bp_d = nc.dram_tensor("bpf", [VQD], fp32, kind="ExternalInput")
    cbt_d = nc.dram_tensor("cbt", [VQD + 1, NQ * K], fp32, kind="ExternalInput")
    cbf_d = nc.dram_tensor("cbf", [NQ * K, VQD], fp32, kind="ExternalInput")

    e_out_d = nc.dram_tensor("e_out", [BPC, EMB, TD], fp32, kind="ExternalOutput")
    idx_out_d = nc.dram_tensor("idx_out", [TPOS, NQ], i32, kind="ExternalOutput")

    relu = mybir.ActivationFunctionType.Relu

    with tile.TileContext(nc) as tc:
        with (
            tc.tile_pool(name="const", bufs=1) as cpool,
            tc.tile_pool(name="x0p", bufs=2) as x0pool,
            tc.tile_pool(name="x1p", bufs=1) as x1pool,
            tc.tile_pool(name="x2p", bufs=1) as x2pool,
            tc.tile_pool(name="work", bufs=3) as wpool,
            tc.tile_pool(name="scr", bufs=3) as spool,
            tc.tile_pool(name="psA", bufs=2, space="PSUM") as psA,
            tc.tile_pool(name="psB", bufs=2, space="PSUM") as psB,
            tc.tile_pool(name="psG", bufs=2, space="PSUM") as psG,
            tc.tile_pool(name="psM", bufs=2, space="PSUM") as psM,
        ):
            # ---- constants ------------------------------------------------
            w1t_sb = [cpool.tile([CIN, K1 * C], f16, tag=f"w1t{s}", name=f"w1t{s}")
                      for s in range(2)]
            w2t_sb = [cpool.tile([128, K1 * 2 * C], f16, tag=f"w2t{s}", name=f"w2t{s}")
                      for s in range(2)]
            wdt_sb = [cpool.tile([128, KD * 2 * EMB], f16, tag=f"wdt{s}", name=f"wdt{s}")
                      for s in range(2)]
            for s in range(2):
                nc.sync.dma_start(
                    w1t_sb[s][:].rearrange("p (k c) -> p k c", k=K1),
                    w1t_d[s].transpose([1, 0, 2]))
                nc.sync.dma_start(
                    w2t_sb[s][:].rearrange("p (k g c) -> p k g c", k=K1, g=2),
                    w2t_d[s].transpose([2, 0, 1, 3]))
                nc.sync.dma_start(
                    wdt_sb[s][:].rearrange("p (k g c) -> p k g c", k=KD, g=2),
                    wdt_d[s].transpose([2, 0, 1, 3]))
            wpt_sb = cpool.tile([EMB, VQD], fp32, tag="wpt")
            nc.sync.dma_start(wpt_sb[:], wpt_d[:])
            b1_sb = cpool.tile([128, 2], fp32, tag="b1")
            b2_sb = cpool.tile([128, 2], fp32, tag="b2")
            nc.sync.dma_start(b1_sb[:], b1_d[:].transpose([1, 0]))
            nc.sync.dma_start(b2_sb[:], b2_d[:].transpose([1, 0]))
            bd_sb = cpool.tile([EMB, 1], fp32, tag="bd")
            nc.sync.dma_start(bd_sb[:], bd_d[:].unsqueeze(1))
            bp_sb = cpool.tile([VQD, 1], fp32, tag="bp")
            nc.sync.dma_start(bp_sb[:], bp_d[:].unsqueeze(1))
            cbt_sb = cpool.tile([VQD + 1, NQ * K], fp32, tag="cbt")
            nc.sync.dma_start(cbt_sb[:], cbt_d[:])
            ident = cpool.tile([128, 128], fp32, tag="ident")
            make_identity(nc, ident[:])

            e_sb = cpool.tile([EMB, TPOS], fp32, tag="e_sb")
            rts = [cpool.tile([VQD + 1, 128], fp32, tag=f"rt{i}", name=f"rt{i}")
                   for i in range(BPC * NLT)]
            idxs = [cpool.tile([128, NQ], i32, tag=f"idx{i}", name=f"idx{i}")
                    for i in range(BPC * NLT)]

            def zfill(ap, n):
                # exact zeros via ACT (out = ident*0.0); n = free elems
                nc.scalar.mul(ap, ident[:, 0:n], 0.0)

            for img in range(BPC):
                # ---- load pre-split input -------------------------------
                x0 = [x0pool.tile([CIN, PADT], f16, tag=f"x0{s}", name=f"x0{s}")
                      for s in range(2)]
                nc.sync.dma_start(x0[0][:], melh_d[img])
                nc.sync.dma_start(x0[1][:], mell_d[img])

                x1 = [[x1pool.tile([128, PADT], f16, tag=f"x1_{c}{s}",
                                   name=f"x1_{c}{s}")
                       for s in range(2)] for c in range(2)]
                x2p = [[x2pool.tile([128, 12, MB], f16, tag=f"x2_{c}{s}",
                                    name=f"x2_{c}{s}")
                        for s in range(2)] for c in range(2)]
                for c in range(2):
                    for s in range(2):
                        zfill(x1[c][s][:, 0:PAD], PAD)
                        zfill(x1[c][s][:, T + PAD:PADT], PAD)
                        zfill(x2p[c][s][:, 0:12, 0], 12)       # m_phys 0
                        zfill(x2p[c][s][:, 4:12, 342], 8)      # t in [4096,4104)

                # ---- conv1: K=80, 5 taps x 3 split terms ----------------
                for h in range(2):
                    for ct in range(NCT):
                        ps = psA.tile([128, CT], fp32, tag="c1")
                        t0 = PAD - 2 + ct * CT
                        n = 0
                        for dk in range(K1):
                            o = dk * C + h * 128
                            for (ws, xs) in ((0, 0), (1, 0), (0, 1)):
                                nc.tensor.matmul(
                                    ps[:], w1t_sb[ws][:, o:o + 128],
                                    x0[xs][:, t0 + dk: t0 + dk + CT],
                                    start=(n == 0), stop=(n == 3 * K1 - 1))
                                n += 1
                        s_t = spool.tile([128, CT], fp32, tag="scr")
                        nc.scalar.activation(s_t[:], ps[:], relu,
                                             bias=b1_sb[:, h:h + 1])
                        dst = slice(PAD + ct * CT, PAD + (ct + 1) * CT)
                        nc.scalar.copy(x1[h][0][:, dst], s_t[:])
                        nc.vector.tensor_sub(x1[h][1][:, dst], s_t[:],
                                             x1[h][0][:, dst])

                # ---- conv2: K=128x2 chunks, 5 taps, 3 split terms -------
                for h in range(2):
                    for ct in range(N2FULL + 1):
                        t0 = ct * CT2
                        w = CT2 if ct < N2FULL else REM2
                        if w == 0:
                            continue
                        ps = psB.tile([128, CT2], fp32, tag="c2")
                        n = 0
                        nmm = 3 * K1 * 2
                        for dk in range(K1):
                            for cc in range(2):
                                o = (dk * 2 + cc) * C + h * 128
                                for (ws, xs) in ((0, 0), (1, 0), (0, 1)):
                                    nc.tensor.matmul(
                                        ps[:, :w], w2t_sb[ws][:, o:o + 128],
                                        x1[cc][xs][:, PAD - 2 + dk + t0:
                                                   PAD - 2 + dk + t0 + w],
                                        start=(n == 0), stop=(n == nmm - 1))
                                    n += 1
                        s_t = spool.tile([128, CT], fp32, tag="scr")
                        nc.scalar.activation(s_t[:, :w], ps[:, :w], relu,
                                             bias=b2_sb[:, h:h + 1])
                        # packed writes: t = t0+j -> (r=t%12, m_phys=t//12+1)
                        m1 = t0 // 12 + 1
                        cnt = w // 12
                        ph = x2p[h][0][:, 0:12, m1:m1 + cnt].transpose([0, 2, 1])
                        pl = x2p[h][1][:, 0:12, m1:m1 + cnt].transpose([0, 2, 1])
                        s3 = s_t[:, :cnt * 12].rearrange(
                            "p (a b) -> p a b", b=12)
                        nc.scalar.copy(ph, s3)
                        nc.vector.tensor_sub(pl, s3, ph)
                        rem = w - cnt * 12
                        if rem:   # last tile tail: t in [4092, 4096)
                            ph2 = x2p[h][0][:, 0:rem, m1 + cnt]
                            pl2 = x2p[h][1][:, 0:rem, m1 + cnt]
                            nc.scalar.copy(ph2, s_t[:, cnt * 12:w])
                            nc.vector.tensor_sub(pl2, s_t[:, cnt * 12:w], ph2)

                # ---- downsample conv (contiguous rhs via packing) -------
                eps = psM.tile([EMB, TD], fp32, tag="misc")
                n = 0
                for j in range(KD):
                    r = (j + 6) % 12
                    m0 = (j + 6) // 12
                    for cc in range(2):
                        o = (j * 2 + cc) * EMB
                        for (ws, xs) in ((0, 0), (1, 0), (0, 1)):
                            nc.tensor.matmul(
                                eps[:], wdt_sb[ws][:, o:o + EMB],
                                x2p[cc][xs][:, r, m0:m0 + TD],
                                start=(n == 0), stop=(n == 6 * KD - 1))
                            n += 1
                nc.vector.tensor_scalar_add(
                    e_sb[:, img * TD:(img + 1) * TD], eps[:], bd_sb[:, 0:1])
                nc.sync.dma_start(e_out_d[img], e_sb[:, img * TD:(img + 1) * TD])

                # ---- wp projection + RVQ for this image -----------------
                for lt in range(NLT):
                    i = img * NLT + lt
                    p0 = img * TD + lt * 128
                    pn = min(128, TD - lt * 128)
                    zps = psM.tile([VQD, 128], fp32, tag="misc")
                    nc.tensor.matmul(zps[:, :pn], wpt_sb[:], e_sb[:, p0:p0 + pn],
                                     start=True, stop=True)
                    nc.gpsimd.memset(rts[i][:], 1.0)
                    nc.vector.tensor_scalar_add(rts[i][0:VQD, :pn], zps[:, :pn],
                                                bp_sb[:, 0:1])
                for q in range(NQ):
                    for lt in range(NLT):
                        i = img * NLT + lt
                        p0 = img * TD + lt * 128
                        pn = min(128, TD - lt * 128)
                        rt = rts[i]
                        gps = psG.tile([128, K], fp32, tag="g")
                        nc.tensor.matmul(gps[:pn], rt[:, :pn],
                                         cbt_sb[:, q * K:(q + 1) * K],
                                         start=True, stop=True)
                        gsb = wpool.tile([128, K], fp32, tag="gsb")
                        nc.scalar.copy(gsb[:pn], gps[:pn])
                        m8 = wpool.tile([128, 8], fp32, tag="m8")
                        nc.vector.max(m8[:pn], gsb[:pn])
                        i8 = wpool.tile([128, 8], u32, tag="i8")
                        nc.vector.max_index(i8[:pn], m8[:pn], gsb[:pn])
                        nc.vector.tensor_copy(idxs[i][:pn, q:q + 1], i8[:pn, 0:1])
                        if q < NQ - 1:
                            ib = wpool.tile([128, 1], u32, tag="ib")
                            nc.vector.tensor_scalar_add(ib[:pn], i8[:pn, 0:1],
                                                        q * K)
                            sel = wpool.tile([128, VQD], fp32, tag="sel")
                            nc.gpsimd.indirect_dma_start(
                                out=sel[:pn], out_offset=None, in_=cbf_d[:],
                                in_offset=bass.IndirectOffsetOnAxis(
                                    ap=ib[:pn, 0:1], axis=0))
                            stps = psM.tile([VQD, 128], fp32, tag="misc")
                            nc.tensor.transpose(stps[:, :pn], sel[:pn, :],
                                                ident[:pn, :pn])
                            nc.vector.tensor_sub(rt[0:VQD, :pn],
                                                 rt[0:VQD, :pn], stps[:, :pn])
                for lt in range(NLT):
                    i = img * NLT + lt
                    p0 = img * TD + lt * 128
                    pn = min(128, TD - lt * 128)
                    nc.sync.dma_start(idx_out_d[p0:p0 + pn, :], idxs[i][:pn, :])

    nc.finalize()
    return nc


def split16(x):
    h = x.astype(np.float16)
    l = (x - h.astype(np.float32)).astype(np.float16)
    return h, l


def host_prep_fp16(inputs):
    f = lambda k: np.asarray(inputs[k], np.float32)
    w1, b1, g1, be1, m1, v1 = (f(k) for k in ("w1", "b1", "g1", "be1", "m1", "v1"))
    w2, b2, g2, be2, m2, v2 = (f(k) for k in ("w2", "b2", "g2", "be2", "m2", "v2"))
    wd, bd, wp, bp, cb = (f(k) for k in ("wd", "bd", "wp", "bp", "codebooks"))

    s1 = (g1 / np.sqrt(v1 + BN_EPS)).astype(np.float32)
    w1f = w1 * s1[:, None, None]
    b1f = ((b1 - m1) * s1 + be1).astype(np.float32)
    s2 = (g2 / np.sqrt(v2 + BN_EPS)).astype(np.float32)
    w2f = w2 * s2[:, None, None]
    b2f = ((b2 - m2) * s2 + be2).astype(np.float32)

    w1t = np.ascontiguousarray(np.transpose(w1f, (2, 1, 0)))
    w2t = np.ascontiguousarray(
        np.transpose(w2f, (2, 1, 0)).reshape(K1, 2, 128, C))
    wdt = np.ascontiguousarray(
        np.transpose(wd, (2, 1, 0)).reshape(KD, 2, 128, EMB))
    wpt = np.ascontiguousarray(wp[:, :, 0].T)

    cbt = np.empty((VQD + 1, NQ * K), np.float32)
    for q in range(NQ):
        cbt[:VQD, q * K:(q + 1) * K] = cb[q].T
        cbt[VQD, q * K:(q + 1) * K] = -0.5 * np.sum(cb[q] * cb[q], axis=-1)
    cbf = np.ascontiguousarray(cb.reshape(NQ * K, VQD))

    shared = {
        "w1t": np.stack(split16(w1t)), "b1f": b1f.reshape(2, 128),
        "w2t": np.stack(split16(w2t)), "b2f": b2f.reshape(2, 128),
        "wdt": np.stack(split16(wdt)), "bdf": bd, "wpt": wpt, "bpf": bp,
        "cbt": cbt, "cbf": cbf,
    }
    return shared


def _host_prep(inputs):
    f = lambda k: np.asarray(inputs[k], np.float32)
    w1, b1, g1, be1, m1, v1 = (f(k) for k in ("w1", "b1", "g1", "be1", "m1", "v1"))
    w2, b2, g2, be2, m2, v2 = (f(k) for k in ("w2", "b2", "g2", "be2", "m2", "v2"))
    wd, bd, wp, bp, cb = (f(k) for k in ("wd", "bd", "wp", "bp", "codebooks"))

    s1 = (g1 / np.sqrt(v1 + BN_EPS)).astype(np.float32)
    w1f = w1 * s1[:, None, None]
    b1f = ((b1 - m1) * s1 + be1).astype(np.float32)
    s2 = (g2 / np.sqrt(v2 + BN_EPS)).astype(np.float32)
    w2f = w2 * s2[:, None, None]
    b2f = ((b2 - m2) * s2 + be2).astype(np.float32)

    w1t = np.ascontiguousarray(np.transpose(w1f, (2, 1, 0)))          # (5,80,256)
    w2t = np.ascontiguousarray(
        np.transpose(w2f, (2, 1, 0)).reshape(K1, 2, 128, C))          # (5,2,128,256)
    wdt = np.ascontiguousarray(
        np.transpose(wd, (2, 1, 0)).reshape(KD, 2, 128, EMB))         # (24,2,128,128)
    wpt = np.ascontiguousarray(wp[:, :, 0].T)                         # (128,16)

    cbt = np.empty((VQD + 1, NQ * K), np.float32)
    for q in range(NQ):
        cbt[:VQD, q * K:(q + 1) * K] = cb[q].T
        cbt[VQD, q * K:(q + 1) * K] = -0.5 * np.sum(cb[q] * cb[q], axis=-1)
    cbf = np.ascontiguousarray(cb.reshape(NQ * K, VQD))

    shared = {
        "w1t": w1t, "b1f": b1f.reshape(2, 128),
        "w2t": w2t, "b2f": b2f.reshape(2, 128),
        "wdt": wdt, "bdf": bd, "wpt": wpt, "bpf": bp,
        "cbt": cbt, "cbf": cbf,
    }
    return shared


def _make_in_maps(inputs, mm_mode):
    mel = np.asarray(inputs["mel_spectrogram"], np.float32)
    mel = np.pad(mel, ((0, 0), (0, 0), (PAD, PAD)))
    if mm_mode == "fp16x2":
        shared = host_prep_fp16(inputs)
        mh, ml = split16(mel)
        return [dict(shared,
                     melh=np.ascontiguousarray(mh[c * BPC:(c + 1) * BPC]),
                     mell=np.ascontiguousarray(ml[c * BPC:(c + 1) * BPC]))
                for c in range(NCORES)]
    shared = _host_prep(inputs)
    return [dict(shared,
                 mel=np.ascontiguousarray(mel[c * BPC:(c + 1) * BPC]))
            for c in range(NCORES)]


def kernel(**inputs):
    from concourse.bass_utils import run_bass_kernel_spmd

    mm_mode = MM_MODE
    if mm_mode not in _PROGRAM_CACHE:
        _PROGRAM_CACHE[mm_mode] = (
            build_fp16() if mm_mode == "fp16x2" else _build_program(mm_mode))
    nc = _PROGRAM_CACHE[mm_mode]

    in_maps = _make_in_maps(inputs, mm_mode)

    res = run_bass_kernel_spmd(nc, in_maps, list(range(NCORES)))
    results = res.results

    e_full = np.empty((B, EMB, TD), np.float32)
    idx_full = np.empty((NQ, B, TD), np.int32)
    for c in range(NCORES):
        e_full[c * BPC:(c + 1) * BPC] = results[c]["e_out"]
        idx = results[c]["idx_out"]                  # (TPOS, NQ)
        idx_full[:, c * BPC:(c + 1) * BPC, :] = (
            idx.T.reshape(NQ, BPC, TD))
    return e_full, idx_full


# revision 14
# speedup vs baseline: 1.7091x; 1.2088x over previous
"""AccentEncoder Trainium2 kernel.

Pipeline per core (2 batch images of the 16, data-parallel over 8 cores):
  conv1 (80->256, k5 p2) + folded BN + ReLU
  conv2 (256->256, k5 p2) + folded BN + ReLU
  downsample conv (256->128, k24 s12 p6)  -> e_a_unq (output 1)
  1x1 conv (128->16)                      -> z
  RVQ: 8 sequential argmin stages         -> indices (output 2)

Convs are computed as sums of shifted matmuls (tap-decomposition), BN is
folded into the conv weights on the host.  The RVQ argmin uses the identity
argmin_k ||r-c_k||^2 == argmax_k (r.c_k - ||c_k||^2/2), computed with one
matmul per (stage, position-tile) whose contraction includes an all-ones row
so the -||c_k||^2/2 term rides along; DVE max/max_index extracts the winner
and an indirect DMA gathers the selected codes for the residual update.
"""

import os
import sys

import numpy as np

try:
    import concourse  # noqa: F401
except ImportError:
    sys.path.insert(0, "/opt/trn_rl_repo")

NCORES = 8
B, CIN, T = 16, 80, 4096
BPC = B // NCORES          # images per core
C = 256                    # conv1/conv2 channels
K1 = 5                     # conv1/conv2 kernel width
STRIDE, KD = 12, 24        # downsample conv
EMB, VQD, NQ, K = 128, 16, 8, 256
TD = (T + 2 * (STRIDE // 2) - KD) // STRIDE + 1   # 341
TPOS = BPC * TD            # positions per core
PAD = 6                    # unified halo (>= conv pad 2 and downsample pad 6)
PADT = T + 2 * PAD         # 4108
BN_EPS = 1e-5
CT = 512                   # conv column-tile width
NCT = T // CT              # 8 column tiles
NPT = (TPOS + 127) // 128  # RVQ position tiles (6)
CINP = 128                 # conv1 contract padded to full PE height (K=80 runs ~3x slow)

# matmul mode: "fp16x2" (split-2 fp16, 3 cyc/row, fp32-class accuracy),
# "fp32" (exact, ~8 cyc/row), "fp32r" (fast but flips VQ indices -- unsafe)
MM_MODE = os.environ.get("ACCENT_MM_MODE", "fp16x2")

_PROGRAM_CACHE = {}


def _build_program(mm_mode):
    from concourse import bacc, bass, mybir, tile
    from concourse.masks import make_identity

    fp32 = mybir.dt.float32
    i32 = mybir.dt.int32
    u32 = mybir.dt.uint32

    # Bacc (not raw Bass): its compile pass moves matmul waits onto ldweights
    # and splits over-limit sync waits, which conv accumulation groups need.
    nc = bacc.Bacc("TRN2")

    fast = mm_mode == "fp32r"
    # conv-path operand dtype: float32r tiles make the producer chain
    # (DMA / ACT epilogue) emit fp32r-rounded data, which the BIR verifier
    # requires for fp32r matmuls. 4x PE throughput at ~1.6e-4 rel err.
    xdt = mybir.dt.float32r if fast else fp32

    mel_d = nc.dram_tensor("mel", [BPC, CIN, PADT], xdt, kind="ExternalInput")
    w1t_d = nc.dram_tensor("w1t", [K1, CIN, C], xdt, kind="ExternalInput")
    b1_d = nc.dram_tensor("b1f", [2, 128], fp32, kind="ExternalInput")
    w2t_d = nc.dram_tensor("w2t", [K1, 2, 128, C], xdt, kind="ExternalInput")
    b2_d = nc.dram_tensor("b2f", [2, 128], fp32, kind="ExternalInput")
    wdt_d = nc.dram_tensor("wdt", [KD, 2, 128, EMB], xdt, kind="ExternalInput")
    bd_d = nc.dram_tensor("bdf", [EMB], fp32, kind="ExternalInput")
    wpt_d = nc.dram_tensor("wpt", [EMB, VQD], fp32, kind="ExternalInput")
    bp_d = nc.dram_tensor("bpf", [VQD], fp32, kind="ExternalInput")
    cbt_d = nc.dram_tensor("cbt", [VQD + 1, NQ * K], fp32, kind="ExternalInput")
    cbf_d = nc.dram_tensor("cbf", [NQ * K, VQD], fp32, kind="ExternalInput")

    e_out_d = nc.dram_tensor("e_out", [BPC, EMB, TD], fp32, kind="ExternalOutput")
    idx_out_d = nc.dram_tensor("idx_out", [TPOS, NQ], i32, kind="ExternalOutput")

    def mm(out, lhsT, rhs, start, stop, force_fp32=False):
        if fast and force_fp32:
            # fp32r matmuls reject strided moving APs (s3d3_mm_fp32r
            # restrictions); the strided-rhs downsample conv runs in fp32.
            lhsT = lhsT.bitcast(fp32)
            rhs = rhs.bitcast(fp32)
        nc.tensor.matmul(out, lhsT, rhs, start=start, stop=stop)

    with tile.TileContext(nc) as tc:
        with (
            tc.tile_pool(name="const", bufs=1) as cpool,
            tc.tile_pool(name="x0p", bufs=1) as x0pool,
            tc.tile_pool(name="x1p", bufs=1) as x1pool,
            tc.tile_pool(name="x2p", bufs=1) as x2pool,
            tc.tile_pool(name="work", bufs=3) as wpool,
            tc.tile_pool(name="psA", bufs=2, space="PSUM") as psA,
            tc.tile_pool(name="psB", bufs=2, space="PSUM") as psB,
            tc.tile_pool(name="psG", bufs=2, space="PSUM") as psG,
            tc.tile_pool(name="psM", bufs=2, space="PSUM") as psM,
        ):
            # ---- static weights / constants -------------------------------
            # single DMA per weight tensor (fan-out over many DMA queues blows
            # the per-instruction sync-wait limit on downstream matmuls)
            w1t_sb = cpool.tile([CIN, K1 * C], xdt, tag="w1t")
            nc.sync.dma_start(
                w1t_sb[:].rearrange("p (k c) -> p k c", k=K1),
                w1t_d[:].transpose([1, 0, 2]))
            w2t_sb = cpool.tile([128, K1 * 2 * C], xdt, tag="w2t")
            nc.sync.dma_start(
                w2t_sb[:].rearrange("p (k g c) -> p k g c", k=K1, g=2),
                w2t_d[:].transpose([2, 0, 1, 3]))
            wdt_sb = cpool.tile([128, KD * 2 * EMB], xdt, tag="wdt")
            nc.sync.dma_start(
                wdt_sb[:].rearrange("p (k g c) -> p k g c", k=KD, g=2),
                wdt_d[:].transpose([2, 0, 1, 3]))
            wpt_sb = cpool.tile([EMB, VQD], fp32, tag="wpt")
            nc.sync.dma_start(wpt_sb[:], wpt_d[:])
            b1_sb = cpool.tile([128, 2], fp32, tag="b1")
            b2_sb = cpool.tile([128, 2], fp32, tag="b2")
            nc.sync.dma_start(b1_sb[:], b1_d[:].transpose([1, 0]))
            nc.sync.dma_start(b2_sb[:], b2_d[:].transpose([1, 0]))
            bd_sb = cpool.tile([EMB, 1], fp32, tag="bd")
            nc.sync.dma_start(bd_sb[:], bd_d[:].unsqueeze(1))
            bp_sb = cpool.tile([VQD, 1], fp32, tag="bp")
            nc.sync.dma_start(bp_sb[:], bp_d[:].unsqueeze(1))
            cbt_sb = cpool.tile([VQD + 1, NQ * K], fp32, tag="cbt")
            nc.sync.dma_start(cbt_sb[:], cbt_d[:])
            ident = cpool.tile([128, 128], fp32, tag="ident")
            make_identity(nc, ident[:])

            e_sb = cpool.tile([EMB, TPOS], fp32, tag="e_sb")
            rts = [cpool.tile([VQD + 1, 128], fp32, tag=f"rt{i}", name=f"rt{i}")
                   for i in range(NPT)]
            idxs = [cpool.tile([128, NQ], i32, tag=f"idx{i}", name=f"idx{i}")
                    for i in range(NPT)]

            relu = mybir.ActivationFunctionType.Relu

            # ---- conv stack, one image at a time --------------------------
            for img in range(BPC):
                x0 = x0pool.tile([CIN, PADT], xdt, tag="x0")
                nc.sync.dma_start(x0[:], mel_d[img])  # host pre-pads halo

                x1c = [x1pool.tile([128, PADT], xdt, tag=f"x1_{c}", name=f"x1_{c}")
                       for c in range(2)]
                x2c = [x2pool.tile([128, PADT], xdt, tag=f"x2_{c}", name=f"x2_{c}")
                       for c in range(2)]
                # pad zeroing on ACT (gpsimd Memset can't emit fp32r, and
                # keeping all x1/x2 writers on one engine keeps matmul waits
                # under the HW sync-wait limit): out = ident*0.0
                for cc in range(2):
                    for xt in (x1c[cc], x2c[cc]):
                        nc.scalar.mul(xt[:, 0:PAD], ident[:, 0:PAD], 0.0)
                        nc.scalar.mul(xt[:, T + PAD:PADT],
                                      ident[:, 0:PAD], 0.0)

                # conv1: K=80, 5 taps, out channels 2 halves
                for h in range(2):
                    for ct in range(NCT):
                        ps = psA.tile([128, CT], fp32, tag="c1")
                        t0 = PAD - 2 + ct * CT
                        for dk in range(K1):
                            mm(ps[:], w1t_sb[:, dk * C + h * 128: dk * C + h * 128 + 128],
                               x0[:, t0 + dk: t0 + dk + CT],
                               start=(dk == 0), stop=(dk == K1 - 1))
                        nc.scalar.activation(
                            x1c[h][:, PAD + ct * CT: PAD + (ct + 1) * CT],
                            ps[:], relu, bias=b1_sb[:, h:h + 1])

                # conv2: K=256 (2 chunks) x 5 taps
                for h in range(2):
                    for ct in range(NCT):
                        ps = psB.tile([128, CT], fp32, tag="c2")
                        t0 = PAD - 2 + ct * CT
                        n = 0
                        for dk in range(K1):
                            for cc in range(2):
                                o = (dk * 2 + cc) * C + h * 128
                                mm(ps[:], w2t_sb[:, o:o + 128],
                                   x1c[cc][:, t0 + dk: t0 + dk + CT],
                                   start=(n == 0), stop=(n == 2 * K1 - 1))
                                n += 1
                        nc.scalar.activation(
                            x2c[h][:, PAD + ct * CT: PAD + (ct + 1) * CT],
                            ps[:], relu, bias=b2_sb[:, h:h + 1])

                # downsample conv: out (128, 341), K=256x24 taps, strided rhs
                eps = psM.tile([EMB, TD], fp32, tag="misc")
                n = 0
                for j in range(KD):
                    for cc in range(2):
                        o = (j * 2 + cc) * EMB
                        mm(eps[:], wdt_sb[:, o:o + EMB],
                           x2c[cc][:, j: j + STRIDE * (TD - 1) + 1: STRIDE],
                           start=(n == 0), stop=(n == 2 * KD - 1),
                           force_fp32=True)
                        n += 1
                nc.vector.tensor_scalar_add(
                    e_sb[:, img * TD:(img + 1) * TD], eps[:], bd_sb[:, 0:1])
                nc.sync.dma_start(e_out_d[img], e_sb[:, img * TD:(img + 1) * TD])

            # ---- 1x1 conv -> residual tiles (VQ dim on partitions) --------
            for i in range(NPT):
                p0 = i * 128
                pn = min(128, TPOS - p0)
                zps = psM.tile([VQD, 128], fp32, tag="misc")
                # z = wp.T @ e  (K=128, M=16, N=pn); keep full fp32
                nc.tensor.matmul(zps[:, :pn], wpt_sb[:], e_sb[:, p0:p0 + pn],
                                 start=True, stop=True)
                # row 16 must be all-ones (rides the -||c||^2/2 term through
                # the score matmul); engines can't address partition 16 alone,
                # so fill the whole tile then overwrite rows 0..15.
                nc.gpsimd.memset(rts[i][:], 1.0)
                nc.vector.tensor_scalar_add(rts[i][0:VQD, :pn], zps[:, :pn],
                                            bp_sb[:, 0:1])

            # ---- RVQ: 8 sequential stages ---------------------------------
            for q in range(NQ):
                for i in range(NPT):
                    p0 = i * 128
                    pn = min(128, TPOS - p0)
                    rt = rts[i]
                    gps = psG.tile([128, K], fp32, tag="g")
                    # scores g[t,k] = r_t . c_k - ||c_k||^2/2 (exact fp32)
                    nc.tensor.matmul(gps[:pn], rt[:, :pn],
                                     cbt_sb[:, q * K:(q + 1) * K],
                                     start=True, stop=True)
                    gsb = wpool.tile([128, K], fp32, tag="gsb")
                    nc.scalar.copy(gsb[:pn], gps[:pn])
                    m8 = wpool.tile([128, 8], fp32, tag="m8")
                    nc.vector.max(m8[:pn], gsb[:pn])
                    i8 = wpool.tile([128, 8], u32, tag="i8")
                    nc.vector.max_index(i8[:pn], m8[:pn], gsb[:pn])
                    nc.vector.tensor_copy(idxs[i][:pn, q:q + 1], i8[:pn, 0:1])
                    if q < NQ - 1:
                        # gather selected codes, transpose, subtract residual
                        ib = wpool.tile([128, 1], u32, tag="ib")
                        nc.vector.tensor_scalar_add(ib[:pn], i8[:pn, 0:1], q * K)
                        sel = wpool.tile([128, VQD], fp32, tag="sel")
                        nc.gpsimd.indirect_dma_start(
                            out=sel[:pn], out_offset=None, in_=cbf_d[:],
                            in_offset=bass.IndirectOffsetOnAxis(
                                ap=ib[:pn, 0:1], axis=0))
                        stps = psM.tile([VQD, 128], fp32, tag="misc")
                        nc.tensor.transpose(stps[:, :pn], sel[:pn, :],
                                            ident[:pn, :pn])
                        nc.vector.tensor_sub(rt[0:VQD, :pn], rt[0:VQD, :pn],
                                             stps[:, :pn])
            for i in range(NPT):
                p0 = i * 128
                pn = min(128, TPOS - p0)
                nc.sync.dma_start(idx_out_d[p0:p0 + pn, :], idxs[i][:pn, :])

    nc.finalize()
    return nc


# ---- fp16 split-2 builder ------------------------------------------------
"""

Every conv matmul runs as 3 fp16 matmuls (xh.wh + xh.wl + xl.wh) where
x = xh + xl is the exact fp16 hi/lo decomposition: ~2^-21 relative error
(fp32-class, verified 4.9e-7 on HW) at 3x1 cyc/row instead of fp32's ~8.

conv2's epilogue writes its output directly in a phase-packed layout
x2p[c, r, m] = x2[c, 12*(m-1)+r], which turns the stride-12 downsample conv
into contiguous-rhs matmuls (strided fp16 moving APs run ~6.5x slower).
RVQ stays exact fp32 so the argmin indices match the reference bit-for-bit.
"""

CT2 = 504                     # conv2 tile width (multiple of 12)
N2FULL = T // CT2             # 8 full tiles
REM2 = T - N2FULL * CT2       # 64
MB = 344                      # packed m size (m_phys = m+1, m in [-1, 342])
NLT = 3                       # RVQ position tiles per image (128,128,85)


def build_fp16():
    from concourse import bacc, bass, mybir, tile
    from concourse.masks import make_identity

    fp32 = mybir.dt.float32
    f16 = mybir.dt.float16
    i32 = mybir.dt.int32
    u32 = mybir.dt.uint32

    nc = bacc.Bacc("TRN2")

    melh_d = nc.dram_tensor("melh", [BPC, CINP, PADT], f16, kind="ExternalInput")
    mell_d = nc.dram_tensor("mell", [BPC, CINP, PADT], f16, kind="ExternalInput")
    w1t_d = nc.dram_tensor("w1t", [2, K1, CINP, C], f16, kind="ExternalInput")
    b1_d = nc.dram_tensor("b1f", [2, 128], fp32, kind="ExternalInput")
    w2t_d = nc.dram_tensor("w2t", [2, K1, 2, 128, C], f16, kind="ExternalInput")
    b2_d = nc.dram_tensor("b2f", [2, 128], fp32, kind="ExternalInput")
    wdt_d = nc.dram_tensor("wdt", [2, KD, 2, 128, EMB], f16, kind="ExternalInput")
    bd_d = nc.dram_tensor("bdf", [EMB], fp32, kind="ExternalInput")
    wpt_d = nc.dram_tensor("wpt", [EMB, VQD], fp32, kind="ExternalInput")
    bp_d = nc.dram_tensor("bpf", [VQD], fp32, kind="ExternalInput")
    cbt_d = nc.dram_tensor("cbt", [VQD + 1, NQ * K], fp32, kind="ExternalInput")
    cbf_d = nc.dram_tensor("cbf", [NQ * K, VQD], fp32, kind="ExternalInput")

    e_out_d = nc.dram_tensor("e_out", [BPC, EMB, TD], fp32, kind="ExternalOutput")
    idx_out_d = nc.dram_tensor("idx_out", [TPOS, NQ], i32, kind="ExternalOutput")

    relu = mybir.ActivationFunctionType.Relu

    with tile.TileContext(nc) as tc:
        with (
            tc.tile_pool(name="const", bufs=1) as cpool,
            tc.tile_pool(name="x0p", bufs=1) as x0pool,
            tc.tile_pool(name="x1p", bufs=1) as x1pool,
            tc.tile_pool(name="x2p", bufs=1) as x2pool,
            tc.tile_pool(name="work", bufs=3) as wpool,
            tc.tile_pool(name="scr", bufs=3) as spool,
            tc.tile_pool(name="psA", bufs=2, space="PSUM") as psA,
            tc.tile_pool(name="psB", bufs=2, space="PSUM") as psB,
            tc.tile_pool(name="psG", bufs=2, space="PSUM") as psG,
            tc.tile_pool(name="psM", bufs=2, space="PSUM") as psM,
        ):
            # ---- input prefetch (before bulk weights: conv1 starts sooner)
            x0_tiles = [
                [x0pool.tile([CINP, PADT], f16, tag=f"x0_{i}{s}",
                             name=f"x0_{i}{s}") for s in range(2)]
                for i in range(BPC)]
            for i in range(BPC):
                nc.sync.dma_start(x0_tiles[i][0][:], melh_d[i])
                nc.sync.dma_start(x0_tiles[i][1][:], mell_d[i])

            # ---- constants ------------------------------------------------
            w1t_sb = [cpool.tile([CINP, K1 * C], f16, tag=f"w1t{s}", name=f"w1t{s}")
                      for s in range(2)]
            w2t_sb = [cpool.tile([128, K1 * 2 * C], f16, tag=f"w2t{s}", name=f"w2t{s}")
                      for s in range(2)]
            wdt_sb = [cpool.tile([128, KD * 2 * EMB], f16, tag=f"wdt{s}", name=f"wdt{s}")
                      for s in range(2)]
            for s in range(2):
                nc.sync.dma_start(
                    w1t_sb[s][:].rearrange("p (k c) -> p k c", k=K1),
                    w1t_d[s].transpose([1, 0, 2]))
                nc.sync.dma_start(
                    w2t_sb[s][:].rearrange("p (k g c) -> p k g c", k=K1, g=2),
                    w2t_d[s].transpose([2, 0, 1, 3]))
                nc.sync.dma_start(
                    wdt_sb[s][:].rearrange("p (k g c) -> p k g c", k=KD, g=2),
                    wdt_d[s].transpose([2, 0, 1, 3]))
            wpt_sb = cpool.tile([EMB, VQD], fp32, tag="wpt")
            nc.sync.dma_start(wpt_sb[:], wpt_d[:])
            b1_sb = cpool.tile([128, 2], fp32, tag="b1")
            b2_sb = cpool.tile([128, 2], fp32, tag="b2")
            nc.sync.dma_start(b1_sb[:], b1_d[:].transpose([1, 0]))
            nc.sync.dma_start(b2_sb[:], b2_d[:].transpose([1, 0]))
            bd_sb = cpool.tile([EMB, 1], fp32, tag="bd")
            nc.sync.dma_start(bd_sb[:], bd_d[:].unsqueeze(1))
            bp_sb = cpool.tile([VQD, 1], fp32, tag="bp")
            nc.sync.dma_start(bp_sb[:], bp_d[:].unsqueeze(1))
            cbt_sb = cpool.tile([VQD + 1, NQ * K], fp32, tag="cbt")
            nc.sync.dma_start(cbt_sb[:], cbt_d[:])
            ident = cpool.tile([128, 128], fp32, tag="ident")
            make_identity(nc, ident[:])

            e_sb = cpool.tile([EMB, TPOS], fp32, tag="e_sb")
            rts = [cpool.tile([VQD + 1, 128], fp32, tag=f"rt{i}", name=f"rt{i}")
                   for i in range(BPC * NLT)]
            idxs = [cpool.tile([128, NQ], i32, tag=f"idx{i}", name=f"idx{i}")
                    for i in range(BPC * NLT)]

            def zfill(ap, n):
                # exact zeros via ACT (out = ident*0.0); n = free elems
                nc.scalar.mul(ap, ident[:, 0:n], 0.0)

            for img in range(BPC):
                # ---- load pre-split input -------------------------------
                x0 = x0_tiles[img]

                x1 = [[x1pool.tile([128, PADT], f16, tag=f"x1_{c}{s}",
                                   name=f"x1_{c}{s}")
                       for s in range(2)] for c in range(2)]
                x2p = [[x2pool.tile([128, 12, MB], f16, tag=f"x2_{c}{s}",
                                    name=f"x2_{c}{s}")
                        for s in range(2)] for c in range(2)]
                for c in range(2):
                    for s in range(2):
                        zfill(x1[c][s][:, 0:PAD], PAD)
                        zfill(x1[c][s][:, T + PAD:PADT], PAD)
                        zfill(x2p[c][s][:, 0:12, 0], 12)       # m_phys 0
                        zfill(x2p[c][s][:, 4:12, 342], 8)      # t in [4096,4104)

                # ---- conv1: K=80, 5 taps x 3 split terms ----------------
                for h in range(2):
                    for ct in range(NCT):
                        ps = psA.tile([128, CT], fp32, tag="c1")
                        t0 = PAD - 2 + ct * CT
                        n = 0
                        for dk in range(K1):
                            o = dk * C + h * 128
                            for (ws, xs) in ((0, 0), (1, 0), (0, 1)):
                                nc.tensor.matmul(
                                    ps[:], w1t_sb[ws][:, o:o + 128],
                                    x0[xs][:, t0 + dk: t0 + dk + CT],
                                    start=(n == 0), stop=(n == 3 * K1 - 1))
                                n += 1
                        s_t = spool.tile([128, CT], fp32, tag="scr")
                        nc.scalar.activation(s_t[:], ps[:], relu,
                                             bias=b1_sb[:, h:h + 1])
                        dst = slice(PAD + ct * CT, PAD + (ct + 1) * CT)
                        nc.scalar.copy(x1[h][0][:, dst], s_t[:])
                        nc.vector.tensor_sub(x1[h][1][:, dst], s_t[:],
                                             x1[h][0][:, dst])

                # ---- conv2: K=128x2 chunks, 5 taps, 3 split terms -------
                for h in range(2):
                    for ct in range(N2FULL + 1):
                        t0 = ct * CT2
                        w = CT2 if ct < N2FULL else REM2
                        if w == 0:
                            continue
                        ps = psB.tile([128, CT2], fp32, tag="c2")
                        n = 0
                        nmm = 3 * K1 * 2
                        for dk in range(K1):
                            for cc in range(2):
                                o = (dk * 2 + cc) * C + h * 128
                                for (ws, xs) in ((0, 0), (1, 0), (0, 1)):
                                    nc.tensor.matmul(
                                        ps[:, :w], w2t_sb[ws][:, o:o + 128],
                                        x1[cc][xs][:, PAD - 2 + dk + t0:
                                                   PAD - 2 + dk + t0 + w],
                                        start=(n == 0), stop=(n == nmm - 1))
                                    n += 1
                        s_t = spool.tile([128, CT], fp32, tag="scr")
                        nc.scalar.activation(s_t[:, :w], ps[:, :w], relu,
                                             bias=b2_sb[:, h:h + 1])
                        # packed writes: t = t0+j -> (r=t%12, m_phys=t//12+1)
                        m1 = t0 // 12 + 1
                        cnt = w // 12
                        ph = x2p[h][0][:, 0:12, m1:m1 + cnt].transpose([0, 2, 1])
                        pl = x2p[h][1][:, 0:12, m1:m1 + cnt].transpose([0, 2, 1])
                        s3 = s_t[:, :cnt * 12].rearrange(
                            "p (a b) -> p a b", b=12)
                        nc.scalar.copy(ph, s3)
                        nc.vector.tensor_sub(pl, s3, ph)
                        rem = w - cnt * 12
                        if rem:   # last tile tail: t in [4092, 4096)
                            ph2 = x2p[h][0][:, 0:rem, m1 + cnt]
                            pl2 = x2p[h][1][:, 0:rem, m1 + cnt]
                            nc.scalar.copy(ph2, s_t[:, cnt * 12:w])
                            nc.vector.tensor_sub(pl2, s_t[:, cnt * 12:w], ph2)

                # ---- downsample conv (contiguous rhs via packing) -------
                eps = psM.tile([EMB, TD], fp32, tag="misc")
                n = 0
                for j in range(KD):
                    r = (j + 6) % 12
                    m0 = (j + 6) // 12
                    for cc in range(2):
                        o = (j * 2 + cc) * EMB
                        for (ws, xs) in ((0, 0), (1, 0), (0, 1)):
                            nc.tensor.matmul(
                                eps[:], wdt_sb[ws][:, o:o + EMB],
                                x2p[cc][xs][:, r, m0:m0 + TD],
                                start=(n == 0), stop=(n == 6 * KD - 1))
                            n += 1
                nc.vector.tensor_scalar_add(
                    e_sb[:, img * TD:(img + 1) * TD], eps[:], bd_sb[:, 0:1])
                nc.sync.dma_start(e_out_d[img], e_sb[:, img * TD:(img + 1) * TD])

                # ---- wp projection + RVQ for this image -----------------
                for lt in range(NLT):
                    i = img * NLT + lt
                    p0 = img * TD + lt * 128
                    pn = min(128, TD - lt * 128)
                    zps = psM.tile([VQD, 128], fp32, tag="misc")
                    nc.tensor.matmul(zps[:, :pn], wpt_sb[:], e_sb[:, p0:p0 + pn],
                                     start=True, stop=True)
                    nc.gpsimd.memset(rts[i][:], 1.0)
                    nc.vector.tensor_scalar_add(rts[i][0:VQD, :pn], zps[:, :pn],
                                                bp_sb[:, 0:1])
                for q in range(NQ):
                    for lt in range(NLT):
                        i = img * NLT + lt
                        p0 = img * TD + lt * 128
                        pn = min(128, TD - lt * 128)
                        rt = rts[i]
                        gps = psG.tile([128, K], fp32, tag="g")
                        nc.tensor.matmul(gps[:pn], rt[:, :pn],
                                         cbt_sb[:, q * K:(q + 1) * K],
                                         start=True, stop=True)
                        m8 = wpool.tile([128, 8], fp32, tag="m8")
                        nc.vector.max(m8[:pn], gps[:pn])
                        i8 = wpool.tile([128, 8], u32, tag="i8")
                        nc.vector.max_index(i8[:pn], m8[:pn], gps[:pn])
                        nc.vector.tensor_copy(idxs[i][:pn, q:q + 1], i8[:pn, 0:1])
                        if q < NQ - 1:
                            ib = wpool.tile([128, 1], u32, tag="ib")
                            nc.vector.tensor_scalar_add(ib[:pn], i8[:pn, 0:1],
                                                        q * K)
                            sel = wpool.tile([128, VQD], fp32, tag="sel")
                            nc.gpsimd.indirect_dma_start(
                                out=sel[:pn], out_offset=None, in_=cbf_d[:],
                                in_offset=bass.IndirectOffsetOnAxis(
                                    ap=ib[:pn, 0:1], axis=0))
                            stps = psM.tile([VQD, 128], fp32, tag="misc")
                            nc.tensor.transpose(stps[:, :pn], sel[:pn, :],
                                                ident[:pn, :pn])
                            nc.vector.tensor_sub(rt[0:VQD, :pn],
                                                 rt[0:VQD, :pn], stps[:, :pn])
                for lt in range(NLT):
                    i = img * NLT + lt
                    p0 = img * TD + lt * 128
                    pn = min(128, TD - lt * 128)
                    nc.sync.dma_start(idx_out_d[p0:p0 + pn, :], idxs[i][:pn, :])

    nc.finalize()
    return nc


def split16(x):
    h = x.astype(np.float16)
    l = (x - h.astype(np.float32)).astype(np.float16)
    return h, l


def host_prep_fp16(inputs):
    f = lambda k: np.asarray(inputs[k], np.float32)
    w1, b1, g1, be1, m1, v1 = (f(k) for k in ("w1", "b1", "g1", "be1", "m1", "v1"))
    w2, b2, g2, be2, m2, v2 = (f(k) for k in ("w2", "b2", "g2", "be2", "m2", "v2"))
    wd, bd, wp, bp, cb = (f(k) for k in ("wd", "bd", "wp", "bp", "codebooks"))

    s1 = (g1 / np.sqrt(v1 + BN_EPS)).astype(np.float32)
    w1f = w1 * s1[:, None, None]
    b1f = ((b1 - m1) * s1 + be1).astype(np.float32)
    s2 = (g2 / np.sqrt(v2 + BN_EPS)).astype(np.float32)
    w2f = w2 * s2[:, None, None]
    b2f = ((b2 - m2) * s2 + be2).astype(np.float32)

    w1t = np.zeros((K1, CINP, C), np.float32)
    w1t[:, :CIN] = np.transpose(w1f, (2, 1, 0))
    w2t = np.ascontiguousarray(
        np.transpose(w2f, (2, 1, 0)).reshape(K1, 2, 128, C))
    wdt = np.ascontiguousarray(
        np.transpose(wd, (2, 1, 0)).reshape(KD, 2, 128, EMB))
    wpt = np.ascontiguousarray(wp[:, :, 0].T)

    cbt = np.empty((VQD + 1, NQ * K), np.float32)
    for q in range(NQ):
        cbt[:VQD, q * K:(q + 1) * K] = cb[q].T
        cbt[VQD, q * K:(q + 1) * K] = -0.5 * np.sum(cb[q] * cb[q], axis=-1)
    cbf = np.ascontiguousarray(cb.reshape(NQ * K, VQD))

    shared = {
        "w1t": np.stack(split16(w1t)), "b1f": b1f.reshape(2, 128),
        "w2t": np.stack(split16(w2t)), "b2f": b2f.reshape(2, 128),
        "wdt": np.stack(split16(wdt)), "bdf": bd, "wpt": wpt, "bpf": bp,
        "cbt": cbt, "cbf": cbf,
    }
    return shared


def _host_prep(inputs):
    f = lambda k: np.asarray(inputs[k], np.float32)
    w1, b1, g1, be1, m1, v1 = (f(k) for k in ("w1", "b1", "g1", "be1", "m1", "v1"))
    w2, b2, g2, be2, m2, v2 = (f(k) for k in ("w2", "b2", "g2", "be2", "m2", "v2"))
    wd, bd, wp, bp, cb = (f(k) for k in ("wd", "bd", "wp", "bp", "codebooks"))

    s1 = (g1 / np.sqrt(v1 + BN_EPS)).astype(np.float32)
    w1f = w1 * s1[:, None, None]
    b1f = ((b1 - m1) * s1 + be1).astype(np.float32)
    s2 = (g2 / np.sqrt(v2 + BN_EPS)).astype(np.float32)
    w2f = w2 * s2[:, None, None]
    b2f = ((b2 - m2) * s2 + be2).astype(np.float32)

    w1t = np.ascontiguousarray(np.transpose(w1f, (2, 1, 0)))          # (5,80,256)
    w2t = np.ascontiguousarray(
        np.transpose(w2f, (2, 1, 0)).reshape(K1, 2, 128, C))          # (5,2,128,256)
    wdt = np.ascontiguousarray(
        np.transpose(wd, (2, 1, 0)).reshape(KD, 2, 128, EMB))         # (24,2,128,128)
    wpt = np.ascontiguousarray(wp[:, :, 0].T)                         # (128,16)

    cbt = np.empty((VQD + 1, NQ * K), np.float32)
    for q in range(NQ):
        cbt[:VQD, q * K:(q + 1) * K] = cb[q].T
        cbt[VQD, q * K:(q + 1) * K] = -0.5 * np.sum(cb[q] * cb[q], axis=-1)
    cbf = np.ascontiguousarray(cb.reshape(NQ * K, VQD))

    shared = {
        "w1t": w1t, "b1f": b1f.reshape(2, 128),
        "w2t": w2t, "b2f": b2f.reshape(2, 128),
        "wdt": wdt, "bdf": bd, "wpt": wpt, "bpf": bp,
        "cbt": cbt, "cbf": cbf,
    }
    return shared


def _make_in_maps(inputs, mm_mode):
    mel = np.asarray(inputs["mel_spectrogram"], np.float32)
    mel = np.pad(mel, ((0, 0), (0, 0), (PAD, PAD)))
    if mm_mode == "fp16x2":
        shared = host_prep_fp16(inputs)
        mel = np.pad(mel, ((0, 0), (0, CINP - CIN), (0, 0)))
        mh, ml = split16(mel)
        return [dict(shared,
                     melh=np.ascontiguousarray(mh[c * BPC:(c + 1) * BPC]),
                     mell=np.ascontiguousarray(ml[c * BPC:(c + 1) * BPC]))
                for c in range(NCORES)]
    shared = _host_prep(inputs)
    return [dict(shared,
                 mel=np.ascontiguousarray(mel[c * BPC:(c + 1) * BPC]))
            for c in range(NCORES)]


def kernel(**inputs):
    from concourse.bass_utils import run_bass_kernel_spmd

    mm_mode = MM_MODE
    if mm_mode not in _PROGRAM_CACHE:
        _PROGRAM_CACHE[mm_mode] = (
            build_fp16() if mm_mode == "fp16x2" else _build_program(mm_mode))
    nc = _PROGRAM_CACHE[mm_mode]

    in_maps = _make_in_maps(inputs, mm_mode)

    res = run_bass_kernel_spmd(nc, in_maps, list(range(NCORES)))
    results = res.results

    e_full = np.empty((B, EMB, TD), np.float32)
    idx_full = np.empty((NQ, B, TD), np.int32)
    for c in range(NCORES):
        e_full[c * BPC:(c + 1) * BPC] = results[c]["e_out"]
        idx = results[c]["idx_out"]                  # (TPOS, NQ)
        idx_full[:, c * BPC:(c + 1) * BPC, :] = (
            idx.T.reshape(NQ, BPC, TD))
    return e_full, idx_full
